# revision 17
# baseline (speedup 1.0000x reference)
# Self-contained Trainium2 Bass kernel for the LN->QKV->sparse-rel-pos-attention->proj block.
#
# Reference computation (B=128, N=256, DIM=512, H=12, KD=32, D=128):
#   xn   = LayerNorm(x) * gamma + beta
#   qkv  = xn @ Wqkv + bqkv ; split q,k,v per head
#   attn = softmax(q k^T / sqrt(KD) + biases[:, bias_idxs])
#   out  = (attn @ v) @ Wproj + bproj
#
# Strategy: pure data-parallel over batch across 8 NeuronCores (16 elems/core).
# Host folds: gamma/beta into Wqkv, 1/sqrt(KD) into Wq, v-bias into bproj,
# and expands exp(biases[:, bias_idxs]) so softmax(S+B) = expS*expB row-normalized.
# Device layouts avoid all transposes except the initial z -> z^T:
#   qk^T [feat, tok] and v [tok, feat] both come from matmuls against z^T;
#   S^T = k q^T has tokens-m on partitions so exp/Z/AV consume it directly;
#   AV gives O^T [head-dim, tok] which is exactly proj's stationary layout.
# Softmax normalizer: Z is computed partition-broadcast directly by a matmul
# with an all-ones [128,128] stationary (same PE cost as a [1,x] row, but the
# result lands replicated across partitions -> no DRAM-roundtrip broadcast),
# then one DVE approx-reciprocal and one fused multiply.
# q/k features pack into 8 chunks of 128 (strips of 32 at bases 0/32/64 --
# the PE requires stationary base partition in {0,32,64}): head h has q in
# chunk h//3, k in chunk 4+h//3, both at strip rows 32*(h%3). Heads are
# processed in strip-pure pairs because interleaving different PE
# tile_positions on one PSUM bank hangs the device.
# PE p-state: full speed (2.4GHz) only after 3us of continuous busy; every
# idle gap drops it to 1.2GHz for the next 3us. The emission order is
# software-pipelined so the PE rarely waits: v-production matmuls interleave
# with S rounds, Z/AV trail S by 2 rounds (covering the exp->expB-mult
# latency on ACT/DVE/GpSimd), next-pair LayerNorm DVE work is spread through
# this pair's rounds, and next-pair transposes cover proj's wait for the
# last normalize.
# x/y ride DMA with 2 tokens per partition line (4KB contiguous packets;
# token-major 2KB lines only reach ~23GB/s) => attention token order is
# position (s*128+q) <-> token 2q+s; expb is permuted to match on the host.
# Weight-constant DMA descriptors are issued from otherwise-idle engines
# (ident/wqk on Activation, the rest on GpSimd; x on Sync) -- issuing one
# rearranged weight DMA costs ~3.6us of the issuing engine's queue, which
# must not block the first LayerNorm Sqrt / zT copies.

import numpy as np

B, N, DIM = 128, 256, 512
H, KD = 12, 32
D = 128
DH = D * H
RES = 16
EPS = 1e-5
NCORES = 8
BPC = B // NCORES

_CACHE = {}

# heads processed in strip-pure pairs ((h%3) equal within each pair), ordered
# so v chunks (c0: heads 0-3, c1: 4-7, c2: 8-11) are needed progressively
HEAD_ORDER = [0, 3, 1, 4, 2, 5, 6, 9, 7, 10, 8, 11]
# qk chunk emission order: rounds 0-2 need q chunks {0,1}, k {4,5}
FC_ORDER = [0, 1, 4, 5, 2, 3, 6, 7]


def _build(bpc, use_bqk, use_bp):
    from contextlib import ExitStack

    import concourse.bacc as bacc
    import concourse.tile as tile
    from concourse import mybir

    f32 = mybir.dt.float32
    f32r = mybir.dt.float32r
    fp16 = mybir.dt.float16
    Alu = mybir.AluOpType
    Act = mybir.ActivationFunctionType

    nc = bacc.Bacc("TRN2", target_bir_lowering=False, debug=False,
                   num_devices=NCORES)

    x_d = nc.dram_tensor("x", [bpc, N, DIM], f32r, kind="ExternalInput").ap()
    wqk_d = nc.dram_tensor("wqk", [DIM, 8 * 128], f32r, kind="ExternalInput").ap()
    wv_d = nc.dram_tensor("wv", [DIM, DH], f32r, kind="ExternalInput").ap()
    wp_d = nc.dram_tensor("wp", [DH, DIM], f32r, kind="ExternalInput").ap()
    expb_d = nc.dram_tensor("expb", [128, 2, H, N], fp16, kind="ExternalInput").ap()
    ones_d = nc.dram_tensor("ones", [128, 128], f32r, kind="ExternalInput").ap()
    ident_d = nc.dram_tensor("ident", [128, 128], f32r, kind="ExternalInput").ap()
    if use_bqk:
        bqk_d = nc.dram_tensor("bqk", [128, 8], f32, kind="ExternalInput").ap()
    if use_bp:
        bp_d = nc.dram_tensor("bp", [DIM], f32, kind="ExternalInput").ap()
    y_d = nc.dram_tensor("y", [bpc, N, DIM], f32, kind="ExternalOutput").ap()

    NP = bpc // 2

    with tile.TileContext(nc) as tc, ExitStack() as ctx:
        consts = ctx.enter_context(tc.tile_pool(name="consts", bufs=1))
        sb_x = ctx.enter_context(tc.tile_pool(name="sb_x", bufs=2))
        sb_zT = ctx.enter_context(tc.tile_pool(name="sb_zT", bufs=2))
        sb_qkT = ctx.enter_context(tc.tile_pool(name="sb_qkT", bufs=2))
        sb_v = ctx.enter_context(tc.tile_pool(name="sb_v", bufs=2))
        sb_pt = ctx.enter_context(tc.tile_pool(name="sb_pt", bufs=3))
        sb_zb = ctx.enter_context(tc.tile_pool(name="sb_zb", bufs=2))
        sb_ot = ctx.enter_context(tc.tile_pool(name="sb_ot", bufs=2))
        sb_yb = ctx.enter_context(tc.tile_pool(name="sb_yb", bufs=2))
        sb_small = ctx.enter_context(tc.tile_pool(name="sb_small", bufs=3))
        ps_work = ctx.enter_context(tc.tile_pool(name="ps_work", bufs=2, space="PSUM"))
        ps_s = ctx.enter_context(tc.tile_pool(name="ps_s", bufs=2, space="PSUM"))
        ps_ot = ctx.enter_context(tc.tile_pool(name="ps_ot", bufs=2, space="PSUM"))
        ps_z = ctx.enter_context(tc.tile_pool(name="ps_z", bufs=2, space="PSUM"))

        x_tiles = {}

        def issue_x(p):
            t = sb_x.tile([128, 4, DIM], f32r, tag="x", bufs=2)
            nc.sync.dma_start(
                out=t.rearrange("q (e two) d -> q e two d", e=2),
                in_=x_d[2 * p:2 * p + 2].rearrange("e (q two) d -> q e two d",
                                                   q=128))
            x_tiles[p] = t

        def x_slot(p, st):
            t = x_tiles[p]
            if isinstance(t, list):
                return t[st // 2][:, st % 2, :]
            return t[:, st, :]

        # first pair's x goes out first on the sync queue, split per element
        # so el0's LayerNorm/transposes start while el1 is still in flight
        x0 = []
        for el in range(2):
            t = sb_x.tile([128, 2, DIM], f32r, tag="x0", bufs=2)
            nc.sync.dma_start(
                out=t,
                in_=x_d[el].rearrange("(q two) d -> q two d", q=128))
            x0.append(t)
        x_tiles[0] = x0

        # ---- constants (Activation DMA queue, first-use order) ----
        ident = consts.tile([128, 128], f32r)
        nc.scalar.dma_start(out=ident, in_=ident_d)
        wqk_sb = consts.tile([128, 4, 8 * 128], f32r)
        nc.scalar.dma_start(out=wqk_sb, in_=wqk_d.rearrange("(kc p) f -> p kc f", p=128))
        # x for pair 1 rides the scalar queue behind wqk (the sync queue is
        # busy with pair 0 and would deliver it only at ~33us)
        x1 = sb_x.tile([128, 4, DIM], f32r, tag="x", bufs=2)
        nc.scalar.dma_start(
            out=x1.rearrange("q (e two) d -> q e two d", e=2),
            in_=x_d[2:4].rearrange("e (q two) d -> q e two d", q=128))
        x_tiles[1] = x1
        wv_sb = consts.tile([128, 4, DH], f32r)
        nc.gpsimd.dma_start(out=wv_sb, in_=wv_d.rearrange("(kc p) f -> p kc f", p=128))
        expb_sb = consts.tile([128, 2, H, N], fp16)
        nc.gpsimd.dma_start(out=expb_sb, in_=expb_d)
        ones_sb = consts.tile([128, 128], f32r)
        nc.gpsimd.dma_start(out=ones_sb, in_=ones_d)
        wp_sb = consts.tile([128, H, DIM], f32r)
        nc.gpsimd.dma_start(out=wp_sb, in_=wp_d.rearrange("(h p) f -> p h f", p=128))
        eps_t = consts.tile([128, 1], f32)
        nc.vector.memset(eps_t, EPS)
        if use_bqk:
            bqk_sb = consts.tile([128, 8], f32)
            nc.gpsimd.dma_start(out=bqk_sb, in_=bqk_d)
        if use_bp:
            bp_sb = consts.tile([128, 1, DIM], f32)
            nc.gpsimd.dma_start(out=bp_sb, in_=bp_d.partition_broadcast(128))

        # ---- LayerNorm pieces (emitted spread through the previous pair) ----
        def ln_stats(p, st, mvt):
            stats = sb_small.tile([128, 6], f32, tag="stats", bufs=5)
            nc.vector.bn_stats(stats, x_slot(p, st))
            nc.vector.bn_aggr(mvt[:, st, :], stats)

        def ln_finish(mvt):
            sig = sb_small.tile([128, 4], f32, tag="sig")
            nc.scalar.activation(sig, mvt[:, :, 1], Act.Sqrt, bias=eps_t,
                                 scale=1.0)
            rsig = sb_small.tile([128, 4], f32, tag="rsig")
            nc.vector.reciprocal(rsig, sig)
            return rsig

        def ln_norm(p, st, mvt, rsig):
            xs = x_slot(p, st)
            nc.vector.tensor_scalar(out=xs, in0=xs,
                                    scalar1=mvt[:, st, 0:1],
                                    scalar2=rsig[:, st:st + 1],
                                    op0=Alu.subtract, op1=Alu.mult)

        def transposes(p, zT=None, els=(0, 1)):
            if zT is None:
                zT = sb_zT.tile([128, 4, 2 * N], f32r, tag="zT")
            for st in range(4):
                el, sl = st // 2, st % 2
                if el not in els:
                    continue
                xs = x_slot(p, st)
                zT_ps = ps_work.tile([128, 512], f32r, tag="work")
                for kc in range(4):
                    nc.tensor.transpose(zT_ps[:, kc * 128:(kc + 1) * 128],
                                        xs[:, kc * 128:(kc + 1) * 128],
                                        ident)
                off = el * N + sl * 128
                nc.scalar.activation(zT[:, :, off:off + 128],
                                     zT_ps.rearrange("p (kc t) -> p kc t", kc=4),
                                     Act.Copy)
            return zT

        def qk_prod(zT, split=False):
            qkT = sb_qkT.tile([128, 8, 2 * N], f32r, tag="qkT", bufs=1)
            spans = [(0, 2 * N)] if not split else [(0, N), (N, N)]
            for off, w in spans:
                for i, fc in enumerate(FC_ORDER):
                    qk_ps = ps_work.tile([128, 512], f32, tag="work")
                    for kc in range(4):
                        nc.tensor.matmul(qk_ps[:, :w],
                                         lhsT=wqk_sb[:, kc, fc * 128:(fc + 1) * 128],
                                         rhs=zT[:, kc, off:off + w],
                                         start=(kc == 0), stop=(kc == 3))
                    if i % 2 == 0:
                        nc.scalar.activation(qkT[:, fc, off:off + w],
                                             qk_ps[:, :w], Act.Copy)
                    else:
                        nc.vector.tensor_copy(out=qkT[:, fc, off:off + w],
                                              in_=qk_ps[:, :w])
            if use_bqk:
                for fc in range(8):
                    nc.vector.tensor_scalar_add(
                        out=qkT[:, fc, :], in0=qkT[:, fc, :],
                        scalar1=bqk_sb[:, fc:fc + 1])
            return qkT

        def v_mm(zT, v_sb, etok, mc, c):
            v_ps = ps_work.tile([128, 512], f32, tag="work")
            for kc in range(4):
                nc.tensor.matmul(
                    v_ps,
                    lhsT=zT[:, kc, etok + mc * 128:etok + (mc + 1) * 128],
                    rhs=wv_sb[:, kc, c * 512:(c + 1) * 512],
                    start=(kc == 0), stop=(kc == 3))
            if (mc + c) % 2 == 0:
                nc.scalar.activation(v_sb[:, mc, c * 512:(c + 1) * 512], v_ps,
                                     Act.Copy)
            else:
                nc.vector.tensor_copy(out=v_sb[:, mc, c * 512:(c + 1) * 512],
                                      in_=v_ps)

        def s_round(qkT, etok, r):
            pt = sb_pt.tile([128, 2, 2, N], f32r, tag="pt")
            for mc in range(2):
                s_ps = ps_s.tile([128, 512], f32, tag="s")
                for hi in range(2):
                    h = HEAD_ORDER[2 * r + hi]
                    qc = h // 3
                    base = (h % 3) * KD
                    nc.tensor.matmul(
                        s_ps[:, hi * N:(hi + 1) * N],
                        lhsT=qkT[base:base + KD, 4 + qc,
                                 etok + mc * 128:etok + (mc + 1) * 128],
                        rhs=qkT[base:base + KD, qc, etok:etok + N],
                        start=True, stop=True)
                nc.scalar.activation(pt[:, mc],
                                     s_ps.rearrange("p (a n) -> p a n", a=2),
                                     Act.Exp)
                eng = nc.gpsimd if (r + mc) % 2 == 0 else nc.vector
                eng.tensor_tensor(out=pt[:, mc], in0=pt[:, mc],
                                  in1=expb_sb[:, mc, 2 * r:2 * r + 2, :],
                                  op=Alu.mult)
            return pt

        def zav_round(pt, v_sb, ot_sb, r):
            zb_ps = ps_z.tile([128, 512], f32, tag="zb")
            for mc in range(2):
                nc.tensor.matmul(zb_ps,
                                 lhsT=ones_sb,
                                 rhs=pt[:, mc, :, :].rearrange("p a n -> p (a n)"),
                                 start=(mc == 0), stop=(mc == 1))
            zr = sb_zb.tile([128, 2, N], f32, tag="zb")
            nc.vector.reciprocal_approx_fast(
                out=zr, in_=zb_ps.rearrange("p (a n) -> p a n", a=2))
            ot_ps = ps_ot.tile([128, 512], f32, tag="otp")
            for hi in range(2):
                h = HEAD_ORDER[2 * r + hi]
                for mc in range(2):
                    nc.tensor.matmul(
                        ot_ps[:, hi * N:(hi + 1) * N],
                        lhsT=v_sb[:, mc, h * 128:(h + 1) * 128],
                        rhs=pt[:, mc, hi, :],
                        start=(mc == 0), stop=(mc == 1))
            # GpSimd cannot read PSUM; normalize stays on DVE
            nc.vector.tensor_tensor(out=ot_sb[:, 2 * r:2 * r + 2, :],
                              in0=ot_ps.rearrange("p (a n) -> p a n", a=2),
                              in1=zr, op=Alu.mult)

        def proj(ot_sb, e):
            yb = sb_yb.tile([128, 2, DIM], f32, tag="yb")
            for nci in range(2):
                y_ps = ps_work.tile([128, 512], f32, tag="work")
                for slot in range(H):
                    nc.tensor.matmul(
                        y_ps,
                        lhsT=ot_sb[:, slot, nci * 128:(nci + 1) * 128],
                        rhs=wp_sb[:, HEAD_ORDER[slot], :],
                        start=(slot == 0), stop=(slot == H - 1))
                if use_bp:
                    nc.vector.tensor_tensor(out=yb[:, nci, :], in0=y_ps,
                                            in1=bp_sb[:, 0, :], op=Alu.add)
                else:
                    nc.scalar.activation(yb[:, nci, :], y_ps, Act.Copy)
            nc.sync.dma_start(
                out=y_d[e].rearrange("(q two) d -> q two d", q=128), in_=yb)

        # ---- prologue: pair 0's LN + transposes, per element so el0's
        # PE work starts while el1's x is still in flight ----
        assert bpc % 2 == 0
        zT = sb_zT.tile([128, 4, 2 * N], f32r, tag="zT")
        for el in range(2):
            mv_e = sb_small.tile([128, 2, 2], f32, tag="mv")
            for s in range(2):
                stats = sb_small.tile([128, 6], f32, tag="stats", bufs=5)
                nc.vector.bn_stats(stats, x0[el][:, s, :])
                nc.vector.bn_aggr(mv_e[:, s, :], stats)
            sig = sb_small.tile([128, 2], f32, tag="sig")
            nc.scalar.activation(sig, mv_e[:, :, 1], Act.Sqrt, bias=eps_t,
                                 scale=1.0)
            rsig = sb_small.tile([128, 2], f32, tag="rsig")
            nc.vector.reciprocal(rsig, sig)
            for s in range(2):
                xs = x0[el][:, s, :]
                nc.vector.tensor_scalar(out=xs, in0=xs,
                                        scalar1=mv_e[:, s, 0:1],
                                        scalar2=rsig[:, s:s + 1],
                                        op0=Alu.subtract, op1=Alu.mult)
            transposes(0, zT=zT, els=(el,))

        for p in range(NP):
            if p + 1 < NP:
                if p + 1 not in x_tiles:
                    issue_x(p + 1)
                mv_n = sb_small.tile([128, 4, 2], f32, tag="mv")
                rs_holder = [None]
            qkT = qk_prod(zT, split=(p == 0))

            prev = None  # (ot_sb, e) of el0 awaiting proj
            for el in range(2):
                e = 2 * p + el
                etok = el * N
                v_sb = sb_v.tile([128, 2, DH], f32r, tag="v")
                ot_sb = sb_ot.tile([128, H, N], f32r, tag="ot")
                pts = {}

                def hook(i):
                    # spread next pair's LN through el0's rounds
                    if el != 0 or p + 1 >= NP:
                        return
                    if i < 4:
                        ln_stats(p + 1, i, mv_n)
                    elif i == 4:
                        rs_holder[0] = ln_finish(mv_n)
                        ln_norm(p + 1, 0, mv_n, rs_holder[0])
                        ln_norm(p + 1, 1, mv_n, rs_holder[0])
                    else:
                        ln_norm(p + 1, 2, mv_n, rs_holder[0])
                        ln_norm(p + 1, 3, mv_n, rs_holder[0])

                v_mm(zT, v_sb, etok, 0, 0)
                v_mm(zT, v_sb, etok, 1, 0)
                if prev is not None:
                    proj(*prev)
                    prev = None
                pts[0] = s_round(qkT, etok, 0)
                hook(0)
                v_mm(zT, v_sb, etok, 0, 1)
                pts[1] = s_round(qkT, etok, 1)
                hook(1)
                v_mm(zT, v_sb, etok, 1, 1)
                pts[2] = s_round(qkT, etok, 2)
                zav_round(pts.pop(0), v_sb, ot_sb, 0)
                hook(2)
                v_mm(zT, v_sb, etok, 0, 2)
                pts[3] = s_round(qkT, etok, 3)
                zav_round(pts.pop(1), v_sb, ot_sb, 1)
                hook(3)
                v_mm(zT, v_sb, etok, 1, 2)
                pts[4] = s_round(qkT, etok, 4)
                zav_round(pts.pop(2), v_sb, ot_sb, 2)
                hook(4)
                pts[5] = s_round(qkT, etok, 5)
                zav_round(pts.pop(3), v_sb, ot_sb, 3)
                hook(5)
                zav_round(pts.pop(4), v_sb, ot_sb, 4)
                zav_round(pts.pop(5), v_sb, ot_sb, 5)

                if el == 0:
                    prev = (ot_sb, e)
                else:
                    # next pair's transposes cover proj's wait for the last
                    # normalize
                    if p + 1 < NP:
                        zT = transposes(p + 1)
                    proj(ot_sb, e)

    nc.compile()
    return nc


def _prepare(x, gamma, beta, Wqkv, bqkv, Wproj, bproj, biases, bias_idxs):
    x = np.ascontiguousarray(np.asarray(x, dtype=np.float32))
    gamma = np.asarray(gamma, dtype=np.float32)
    beta = np.asarray(beta, dtype=np.float32)
    Wqkv = np.asarray(Wqkv, dtype=np.float32)
    bqkv = np.asarray(bqkv, dtype=np.float32)
    Wproj = np.asarray(Wproj, dtype=np.float32)
    bproj = np.asarray(bproj, dtype=np.float32)
    biases = np.asarray(biases, dtype=np.float32)
    bias_idxs = np.asarray(bias_idxs)

    s = np.float32(KD ** -0.5)
    Wg = Wqkv * gamma[:, None]
    bfull = beta @ Wqkv + bqkv
    Wr = Wg.reshape(DIM, H, 64 + D)
    br = bfull.reshape(H, 64 + D)
    # feature layout (see kernel comment): head h -> strip h%3; q in chunk
    # h//3, k in chunk 4 + h//3.
    wqk = np.zeros((DIM, 8, 128), dtype=np.float32)
    bqk = np.zeros((8, 128), dtype=np.float32)
    for h in range(H):
        qc, base = h // 3, (h % 3) * KD
        wqk[:, qc, base:base + KD] = Wr[:, h, 0:KD] * s
        wqk[:, 4 + qc, base:base + KD] = Wr[:, h, KD:2 * KD]
        bqk[qc, base:base + KD] = br[h, 0:KD] * s
        bqk[4 + qc, base:base + KD] = br[h, KD:2 * KD]
    wqk = np.ascontiguousarray(wqk.reshape(DIM, 8 * 128))
    wv = np.ascontiguousarray(Wr[:, :, 2 * KD:].reshape(DIM, DH))
    bv = br[:, 2 * KD:].reshape(DH)
    bp = bproj + bv @ Wproj
    expb = np.exp(biases[:, bias_idxs])  # [H, N, N]
    # token positions are interleaved 2-per-partition: pos (s*128+q) <-> token
    # 2q+s; permute both attention axes to match, then reorder heads to the
    # kernel's processing order
    perm = np.arange(N).reshape(128, 2).T.reshape(-1)  # pos -> token
    expb_p = expb[HEAD_ORDER][:, perm][:, :, perm]
    expb_t = np.ascontiguousarray(
        expb_p.reshape(H, 2, 128, N).transpose(2, 1, 0, 3)).astype(np.float16)

    use_bqk = bool(np.abs(bqk).max() > 0)
    use_bp = bool(np.abs(bp).max() > 0)
    bqk_t = np.ascontiguousarray(bqk.T)  # [128, 8]

    common = {"wqk": wqk, "wv": wv, "wp": np.ascontiguousarray(Wproj),
              "expb": expb_t, "ones": np.ones((128, 128), dtype=np.float32),
              "ident": np.eye(128, dtype=np.float32)}
    if use_bqk:
        common["bqk"] = bqk_t
    if use_bp:
        common["bp"] = np.ascontiguousarray(bp)
    in_maps = []
    for c in range(NCORES):
        m = dict(common)
        m["x"] = np.ascontiguousarray(x[c * BPC:(c + 1) * BPC])
        in_maps.append(m)
    return in_maps, use_bqk, use_bp


def run(inputs, trace=False, **run_kwargs):
    from concourse.bass_utils import run_bass_kernel_spmd

    in_maps, use_bqk, use_bp = _prepare(**inputs)
    key = (BPC, use_bqk, use_bp)
    if key not in _CACHE:
        _CACHE[key] = _build(*key)
    nc = _CACHE[key]
    res = run_bass_kernel_spmd(nc, in_maps, core_ids=list(range(NCORES)),
                               trace=trace, **run_kwargs)
    y = np.concatenate([res.results[c]["y"] for c in range(NCORES)], axis=0)
    return y, res


def kernel(**inputs):
    y, _ = run(inputs)
    return y


# revision 18
# speedup vs baseline: 1.2021x; 1.2021x over previous
# Self-contained Trainium2 Bass kernel for the LN->QKV->sparse-rel-pos-attention->proj block.
#
# Reference computation (B=128, N=256, DIM=512, H=12, KD=32, D=128):
#   xn   = LayerNorm(x) * gamma + beta
#   qkv  = xn @ Wqkv + bqkv ; split q,k,v per head
#   attn = softmax(q k^T / sqrt(KD) + biases[:, bias_idxs])
#   out  = (attn @ v) @ Wproj + bproj
#
# Strategy: pure data-parallel over batch across 8 NeuronCores (16 elems/core).
# Host folds: gamma/beta into Wqkv, 1/sqrt(KD) into Wq, v-bias into bproj,
# and expands exp(biases[:, bias_idxs]) so softmax(S+B) = expS*expB row-normalized.
# Device layouts avoid all transposes except the initial z -> z^T:
#   qk^T [feat, tok] and v [tok, feat] both come from matmuls against z^T;
#   S^T = k q^T has tokens-m on partitions so exp/Z/AV consume it directly;
#   AV gives O^T [head-dim, tok] which is exactly proj's stationary layout.
# Softmax normalizer: Z is computed partition-broadcast directly by a matmul
# with an all-ones [128,128] stationary (same PE cost as a [1,x] row, but the
# result lands replicated across partitions -> no DRAM-roundtrip broadcast),
# then one DVE approx-reciprocal and one fused multiply.
# q/k features pack into 8 chunks of 128 (strips of 32 at bases 0/32/64 --
# the PE requires stationary base partition in {0,32,64}): head h has q in
# chunk h//3, k in chunk 4+h//3, both at strip rows 32*(h%3). Heads are
# processed in strip-pure pairs because interleaving different PE
# tile_positions on one PSUM bank hangs the device.
# PE p-state: full speed (2.4GHz) only after 3us of continuous busy; every
# idle gap drops it to 1.2GHz for the next 3us. The emission order is
# software-pipelined so the PE rarely waits: v-production matmuls interleave
# with S rounds, Z/AV trail S by 2 rounds (covering the exp->expB-mult
# latency on ACT/DVE/GpSimd), next-pair LayerNorm DVE work is spread through
# this pair's rounds, and next-pair transposes cover proj's wait for the
# last normalize.
# x/y ride DMA with 2 tokens per partition line (4KB contiguous packets;
# token-major 2KB lines only reach ~23GB/s) => attention token order is
# position (s*128+q) <-> token 2q+s; expb is permuted to match on the host.
# Weight-constant DMA descriptors are issued from otherwise-idle engines
# (ident/wqk on Activation, the rest on GpSimd; x on Sync) -- issuing one
# rearranged weight DMA costs ~3.6us of the issuing engine's queue, which
# must not block the first LayerNorm Sqrt / zT copies.

import numpy as np

B, N, DIM = 128, 256, 512
H, KD = 12, 32
D = 128
DH = D * H
RES = 16
EPS = 1e-5
NCORES = 8
BPC = B // NCORES

_CACHE = {}

# heads processed in strip-pure pairs ((h%3) equal within each pair), ordered
# so v chunks (c0: heads 0-3, c1: 4-7, c2: 8-11) are needed progressively
HEAD_ORDER = [0, 3, 1, 4, 2, 5, 6, 9, 7, 10, 8, 11]
# qk chunk emission order: rounds 0-2 need q chunks {0,1}, k {4,5}
FC_ORDER = [0, 1, 4, 5, 2, 3, 6, 7]


def _build(bpc, use_bqk, use_bp):
    from contextlib import ExitStack

    import concourse.bacc as bacc
    import concourse.tile as tile
    from concourse import mybir

    f32 = mybir.dt.float32
    f32r = mybir.dt.float32r
    fp16 = mybir.dt.float16
    Alu = mybir.AluOpType
    Act = mybir.ActivationFunctionType

    nc = bacc.Bacc("TRN2", target_bir_lowering=False, debug=False,
                   num_devices=NCORES)

    x_d = nc.dram_tensor("x", [bpc, N, DIM], f32r, kind="ExternalInput").ap()
    wqk_d = nc.dram_tensor("wqk", [DIM, 8 * 128], f32r, kind="ExternalInput").ap()
    wv_d = nc.dram_tensor("wv", [DIM, DH], f32r, kind="ExternalInput").ap()
    wp_d = nc.dram_tensor("wp", [DH, DIM], f32r, kind="ExternalInput").ap()
    expb_d = nc.dram_tensor("expb", [128, 2, H, N], fp16, kind="ExternalInput").ap()
    ones_d = nc.dram_tensor("ones", [128, 128], f32r, kind="ExternalInput").ap()
    ident_d = nc.dram_tensor("ident", [128, 128], f32r, kind="ExternalInput").ap()
    if use_bqk:
        bqk_d = nc.dram_tensor("bqk", [128, 8], f32, kind="ExternalInput").ap()
    if use_bp:
        bp_d = nc.dram_tensor("bp", [DIM], f32, kind="ExternalInput").ap()
    y_d = nc.dram_tensor("y", [bpc, N, DIM], f32, kind="ExternalOutput").ap()

    NP = bpc // 2

    with tile.TileContext(nc) as tc, ExitStack() as ctx:
        consts = ctx.enter_context(tc.tile_pool(name="consts", bufs=1))
        sb_x = ctx.enter_context(tc.tile_pool(name="sb_x", bufs=2))
        sb_zT = ctx.enter_context(tc.tile_pool(name="sb_zT", bufs=2))
        sb_qkT = ctx.enter_context(tc.tile_pool(name="sb_qkT", bufs=2))
        sb_v = ctx.enter_context(tc.tile_pool(name="sb_v", bufs=2))
        sb_pt = ctx.enter_context(tc.tile_pool(name="sb_pt", bufs=3))
        sb_zb = ctx.enter_context(tc.tile_pool(name="sb_zb", bufs=2))
        sb_ot = ctx.enter_context(tc.tile_pool(name="sb_ot", bufs=2))
        sb_yb = ctx.enter_context(tc.tile_pool(name="sb_yb", bufs=2))
        sb_small = ctx.enter_context(tc.tile_pool(name="sb_small", bufs=3))
        ps_work = ctx.enter_context(tc.tile_pool(name="ps_work", bufs=2, space="PSUM"))
        ps_s = ctx.enter_context(tc.tile_pool(name="ps_s", bufs=2, space="PSUM"))
        ps_ot = ctx.enter_context(tc.tile_pool(name="ps_ot", bufs=2, space="PSUM"))
        ps_z = ctx.enter_context(tc.tile_pool(name="ps_z", bufs=2, space="PSUM"))

        x_tiles = {}

        def issue_x(p):
            t = sb_x.tile([128, 4, DIM], f32r, tag="x", bufs=2)
            nc.sync.dma_start(
                out=t.rearrange("q (e two) d -> q e two d", e=2),
                in_=x_d[2 * p:2 * p + 2].rearrange("e (q two) d -> q e two d",
                                                   q=128))
            x_tiles[p] = t

        def x_slot(p, st):
            t = x_tiles[p]
            if isinstance(t, list):
                return t[st // 2][:, st % 2, :]
            return t[:, st, :]

        # first pair's x goes out first on the sync queue, split per element
        # so el0's LayerNorm/transposes start while el1 is still in flight
        x0 = []
        for el in range(2):
            t = sb_x.tile([128, 2, DIM], f32r, tag="x0", bufs=2)
            nc.sync.dma_start(
                out=t,
                in_=x_d[el].rearrange("(q two) d -> q two d", q=128))
            x0.append(t)
        x_tiles[0] = x0

        # ---- constants (Activation DMA queue, first-use order) ----
        ident = consts.tile([128, 128], f32r)
        nc.scalar.dma_start(out=ident, in_=ident_d)
        wqk_sb = consts.tile([128, 4, 8 * 128], f32r)
        nc.scalar.dma_start(out=wqk_sb, in_=wqk_d.rearrange("(kc p) f -> p kc f", p=128))
        wv_sb = consts.tile([128, 4, DH], f32r)
        nc.gpsimd.dma_start(out=wv_sb, in_=wv_d.rearrange("(kc p) f -> p kc f", p=128))
        expb_sb = consts.tile([128, 2, H, N], fp16)
        nc.gpsimd.dma_start(out=expb_sb, in_=expb_d)
        ones_sb = consts.tile([128, 128], f32r)
        nc.gpsimd.dma_start(out=ones_sb, in_=ones_d)
        wp_sb = consts.tile([128, H, DIM], f32r)
        nc.gpsimd.dma_start(out=wp_sb, in_=wp_d.rearrange("(h p) f -> p h f", p=128))
        eps_t = consts.tile([128, 1], f32)
        nc.vector.memset(eps_t, EPS)
        if use_bqk:
            bqk_sb = consts.tile([128, 8], f32)
            nc.gpsimd.dma_start(out=bqk_sb, in_=bqk_d)
        if use_bp:
            bp_sb = consts.tile([128, 1, DIM], f32)
            nc.gpsimd.dma_start(out=bp_sb, in_=bp_d.partition_broadcast(128))

        # ---- LayerNorm pieces (emitted spread through the previous pair) ----
        def ln_stats(p, st, mvt):
            stats = sb_small.tile([128, 6], f32, tag="stats", bufs=5)
            nc.vector.bn_stats(stats, x_slot(p, st))
            nc.vector.bn_aggr(mvt[:, st, :], stats)

        def ln_finish(mvt):
            sig = sb_small.tile([128, 4], f32, tag="sig")
            nc.scalar.activation(sig, mvt[:, :, 1], Act.Sqrt, bias=eps_t,
                                 scale=1.0)
            rsig = sb_small.tile([128, 4], f32, tag="rsig")
            nc.vector.reciprocal(rsig, sig)
            return rsig

        def ln_norm(p, st, mvt, rsig):
            xs = x_slot(p, st)
            nc.vector.tensor_scalar(out=xs, in0=xs,
                                    scalar1=mvt[:, st, 0:1],
                                    scalar2=rsig[:, st:st + 1],
                                    op0=Alu.subtract, op1=Alu.mult)

        def transposes(p, zT=None, els=(0, 1)):
            if zT is None:
                zT = sb_zT.tile([128, 4, 2 * N], f32r, tag="zT")
            for st in range(4):
                el, sl = st // 2, st % 2
                if el not in els:
                    continue
                xs = x_slot(p, st)
                zT_ps = ps_work.tile([128, 512], f32r, tag="work")
                for kc in range(4):
                    nc.tensor.transpose(zT_ps[:, kc * 128:(kc + 1) * 128],
                                        xs[:, kc * 128:(kc + 1) * 128],
                                        ident)
                off = el * N + sl * 128
                nc.scalar.activation(zT[:, :, off:off + 128],
                                     zT_ps.rearrange("p (kc t) -> p kc t", kc=4),
                                     Act.Copy)
            return zT

        def qk_prod(zT, split=False):
            qkT = sb_qkT.tile([128, 8, 2 * N], f32r, tag="qkT", bufs=1)
            spans = [(0, 2 * N)] if not split else [(0, N), (N, N)]
            for off, w in spans:
                for i, fc in enumerate(FC_ORDER):
                    qk_ps = ps_work.tile([128, 512], f32, tag="work")
                    for kc in range(4):
                        nc.tensor.matmul(qk_ps[:, :w],
                                         lhsT=wqk_sb[:, kc, fc * 128:(fc + 1) * 128],
                                         rhs=zT[:, kc, off:off + w],
                                         start=(kc == 0), stop=(kc == 3))
                    if i % 2 == 0:
                        nc.scalar.activation(qkT[:, fc, off:off + w],
                                             qk_ps[:, :w], Act.Copy)
                    else:
                        nc.vector.tensor_copy(out=qkT[:, fc, off:off + w],
                                              in_=qk_ps[:, :w])
            if use_bqk:
                for fc in range(8):
                    nc.vector.tensor_scalar_add(
                        out=qkT[:, fc, :], in0=qkT[:, fc, :],
                        scalar1=bqk_sb[:, fc:fc + 1])
            return qkT

        def v_mm(zT, v_sb, etok, mc, c):
            v_ps = ps_work.tile([128, 512], f32, tag="work")
            for kc in range(4):
                nc.tensor.matmul(
                    v_ps,
                    lhsT=zT[:, kc, etok + mc * 128:etok + (mc + 1) * 128],
                    rhs=wv_sb[:, kc, c * 512:(c + 1) * 512],
                    start=(kc == 0), stop=(kc == 3))
            if (mc + c) % 2 == 0:
                nc.scalar.activation(v_sb[:, mc, c * 512:(c + 1) * 512], v_ps,
                                     Act.Copy)
            else:
                nc.vector.tensor_copy(out=v_sb[:, mc, c * 512:(c + 1) * 512],
                                      in_=v_ps)

        def s_round(qkT, etok, r):
            pt = sb_pt.tile([128, 2, 2, N], f32r, tag="pt")
            for mc in range(2):
                s_ps = ps_s.tile([128, 512], f32, tag="s")
                for hi in range(2):
                    h = HEAD_ORDER[2 * r + hi]
                    qc = h // 3
                    base = (h % 3) * KD
                    nc.tensor.matmul(
                        s_ps[:, hi * N:(hi + 1) * N],
                        lhsT=qkT[base:base + KD, 4 + qc,
                                 etok + mc * 128:etok + (mc + 1) * 128],
                        rhs=qkT[base:base + KD, qc, etok:etok + N],
                        start=True, stop=True)
                nc.scalar.activation(pt[:, mc],
                                     s_ps.rearrange("p (a n) -> p a n", a=2),
                                     Act.Exp)
                eng = nc.gpsimd if (r + mc) % 2 == 0 else nc.vector
                eng.tensor_tensor(out=pt[:, mc], in0=pt[:, mc],
                                  in1=expb_sb[:, mc, 2 * r:2 * r + 2, :],
                                  op=Alu.mult)
            return pt

        def zav_round(pt, v_sb, ot_sb, r):
            zb_ps = ps_z.tile([128, 512], f32, tag="zb")
            for mc in range(2):
                nc.tensor.matmul(zb_ps,
                                 lhsT=ones_sb,
                                 rhs=pt[:, mc, :, :].rearrange("p a n -> p (a n)"),
                                 start=(mc == 0), stop=(mc == 1))
            zr = sb_zb.tile([128, 2, N], f32, tag="zb")
            nc.vector.reciprocal_approx_fast(
                out=zr, in_=zb_ps.rearrange("p (a n) -> p a n", a=2))
            ot_ps = ps_ot.tile([128, 512], f32, tag="otp")
            for hi in range(2):
                h = HEAD_ORDER[2 * r + hi]
                for mc in range(2):
                    nc.tensor.matmul(
                        ot_ps[:, hi * N:(hi + 1) * N],
                        lhsT=v_sb[:, mc, h * 128:(h + 1) * 128],
                        rhs=pt[:, mc, hi, :],
                        start=(mc == 0), stop=(mc == 1))
            # GpSimd cannot read PSUM; normalize stays on DVE
            nc.vector.tensor_tensor(out=ot_sb[:, 2 * r:2 * r + 2, :],
                              in0=ot_ps.rearrange("p (a n) -> p a n", a=2),
                              in1=zr, op=Alu.mult)

        def proj(ot_sb, e):
            yb = sb_yb.tile([128, 2, DIM], f32, tag="yb")
            for nci in range(2):
                y_ps = ps_work.tile([128, 512], f32, tag="work")
                for slot in range(H):
                    nc.tensor.matmul(
                        y_ps,
                        lhsT=ot_sb[:, slot, nci * 128:(nci + 1) * 128],
                        rhs=wp_sb[:, HEAD_ORDER[slot], :],
                        start=(slot == 0), stop=(slot == H - 1))
                if use_bp:
                    nc.vector.tensor_tensor(out=yb[:, nci, :], in0=y_ps,
                                            in1=bp_sb[:, 0, :], op=Alu.add)
                else:
                    nc.scalar.activation(yb[:, nci, :], y_ps, Act.Copy)
            nc.sync.dma_start(
                out=y_d[e].rearrange("(q two) d -> q two d", q=128), in_=yb)

        # ---- prologue: pair 0's LN + transposes, per element so el0's
        # PE work starts while el1's x is still in flight ----
        assert bpc % 2 == 0
        zT = sb_zT.tile([128, 4, 2 * N], f32r, tag="zT")
        for el in range(2):
            mv_e = sb_small.tile([128, 2, 2], f32, tag="mv")
            for s in range(2):
                stats = sb_small.tile([128, 6], f32, tag="stats", bufs=5)
                nc.vector.bn_stats(stats, x0[el][:, s, :])
                nc.vector.bn_aggr(mv_e[:, s, :], stats)
            sig = sb_small.tile([128, 2], f32, tag="sig")
            nc.scalar.activation(sig, mv_e[:, :, 1], Act.Sqrt, bias=eps_t,
                                 scale=1.0)
            rsig = sb_small.tile([128, 2], f32, tag="rsig")
            nc.vector.reciprocal(rsig, sig)
            for s in range(2):
                xs = x0[el][:, s, :]
                nc.vector.tensor_scalar(out=xs, in0=xs,
                                        scalar1=mv_e[:, s, 0:1],
                                        scalar2=rsig[:, s:s + 1],
                                        op0=Alu.subtract, op1=Alu.mult)
            transposes(0, zT=zT, els=(el,))

        for p in range(NP):
            if p + 1 < NP:
                if p + 1 not in x_tiles:
                    issue_x(p + 1)
                mv_n = sb_small.tile([128, 4, 2], f32, tag="mv")
                rs_holder = [None]
            qkT = qk_prod(zT, split=(p == 0))

            prev = None  # (ot_sb, e) of el0 awaiting proj
            for el in range(2):
                e = 2 * p + el
                etok = el * N
                v_sb = sb_v.tile([128, 2, DH], f32r, tag="v")
                ot_sb = sb_ot.tile([128, H, N], f32r, tag="ot")
                pts = {}

                def hook(i):
                    # spread next pair's LN through el0's rounds. The tile
                    # scheduler orders engine queues by SIMULATED ready time
                    # (it does not respect emission order); pair 1's x lands
                    # much later than the sim thinks, so floor pair-1 LN deep
                    # into the pair-0 queue or it head-of-line-blocks the
                    # prologue's reciprocals on the DVE.
                    if el != 0 or p + 1 >= NP:
                        return
                    from contextlib import nullcontext
                    wait = tc.tile_wait_until(0.04) if p == 0 else nullcontext()
                    with wait:
                        if i < 4:
                            ln_stats(p + 1, i, mv_n)
                        elif i == 4:
                            rs_holder[0] = ln_finish(mv_n)
                            ln_norm(p + 1, 0, mv_n, rs_holder[0])
                            ln_norm(p + 1, 1, mv_n, rs_holder[0])
                        else:
                            ln_norm(p + 1, 2, mv_n, rs_holder[0])
                            ln_norm(p + 1, 3, mv_n, rs_holder[0])

                v_mm(zT, v_sb, etok, 0, 0)
                v_mm(zT, v_sb, etok, 1, 0)
                if prev is not None:
                    proj(*prev)
                    prev = None
                pts[0] = s_round(qkT, etok, 0)
                hook(0)
                v_mm(zT, v_sb, etok, 0, 1)
                pts[1] = s_round(qkT, etok, 1)
                hook(1)
                v_mm(zT, v_sb, etok, 1, 1)
                pts[2] = s_round(qkT, etok, 2)
                zav_round(pts.pop(0), v_sb, ot_sb, 0)
                hook(2)
                v_mm(zT, v_sb, etok, 0, 2)
                pts[3] = s_round(qkT, etok, 3)
                zav_round(pts.pop(1), v_sb, ot_sb, 1)
                hook(3)
                v_mm(zT, v_sb, etok, 1, 2)
                pts[4] = s_round(qkT, etok, 4)
                zav_round(pts.pop(2), v_sb, ot_sb, 2)
                hook(4)
                pts[5] = s_round(qkT, etok, 5)
                zav_round(pts.pop(3), v_sb, ot_sb, 3)
                hook(5)
                zav_round(pts.pop(4), v_sb, ot_sb, 4)
                zav_round(pts.pop(5), v_sb, ot_sb, 5)

                if el == 0:
                    prev = (ot_sb, e)
                else:
                    # next pair's transposes cover proj's wait for the last
                    # normalize
                    if p + 1 < NP:
                        zT = transposes(p + 1)
                    proj(ot_sb, e)

    nc.compile()
    return nc


def _prepare(x, gamma, beta, Wqkv, bqkv, Wproj, bproj, biases, bias_idxs):
    x = np.ascontiguousarray(np.asarray(x, dtype=np.float32))
    gamma = np.asarray(gamma, dtype=np.float32)
    beta = np.asarray(beta, dtype=np.float32)
    Wqkv = np.asarray(Wqkv, dtype=np.float32)
    bqkv = np.asarray(bqkv, dtype=np.float32)
    Wproj = np.asarray(Wproj, dtype=np.float32)
    bproj = np.asarray(bproj, dtype=np.float32)
    biases = np.asarray(biases, dtype=np.float32)
    bias_idxs = np.asarray(bias_idxs)

    s = np.float32(KD ** -0.5)
    Wg = Wqkv * gamma[:, None]
    bfull = beta @ Wqkv + bqkv
    Wr = Wg.reshape(DIM, H, 64 + D)
    br = bfull.reshape(H, 64 + D)
    # feature layout (see kernel comment): head h -> strip h%3; q in chunk
    # h//3, k in chunk 4 + h//3.
    wqk = np.zeros((DIM, 8, 128), dtype=np.float32)
    bqk = np.zeros((8, 128), dtype=np.float32)
    for h in range(H):
        qc, base = h // 3, (h % 3) * KD
        wqk[:, qc, base:base + KD] = Wr[:, h, 0:KD] * s
        wqk[:, 4 + qc, base:base + KD] = Wr[:, h, KD:2 * KD]
        bqk[qc, base:base + KD] = br[h, 0:KD] * s
        bqk[4 + qc, base:base + KD] = br[h, KD:2 * KD]
    wqk = np.ascontiguousarray(wqk.reshape(DIM, 8 * 128))
    wv = np.ascontiguousarray(Wr[:, :, 2 * KD:].reshape(DIM, DH))
    bv = br[:, 2 * KD:].reshape(DH)
    bp = bproj + bv @ Wproj
    expb = np.exp(biases[:, bias_idxs])  # [H, N, N]
    # token positions are interleaved 2-per-partition: pos (s*128+q) <-> token
    # 2q+s; permute both attention axes to match, then reorder heads to the
    # kernel's processing order
    perm = np.arange(N).reshape(128, 2).T.reshape(-1)  # pos -> token
    expb_p = expb[HEAD_ORDER][:, perm][:, :, perm]
    expb_t = np.ascontiguousarray(
        expb_p.reshape(H, 2, 128, N).transpose(2, 1, 0, 3)).astype(np.float16)

    use_bqk = bool(np.abs(bqk).max() > 0)
    use_bp = bool(np.abs(bp).max() > 0)
    bqk_t = np.ascontiguousarray(bqk.T)  # [128, 8]

    common = {"wqk": wqk, "wv": wv, "wp": np.ascontiguousarray(Wproj),
              "expb": expb_t, "ones": np.ones((128, 128), dtype=np.float32),
              "ident": np.eye(128, dtype=np.float32)}
    if use_bqk:
        common["bqk"] = bqk_t
    if use_bp:
        common["bp"] = np.ascontiguousarray(bp)
    in_maps = []
    for c in range(NCORES):
        m = dict(common)
        m["x"] = np.ascontiguousarray(x[c * BPC:(c + 1) * BPC])
        in_maps.append(m)
    return in_maps, use_bqk, use_bp


def run(inputs, trace=False, **run_kwargs):
    from concourse.bass_utils import run_bass_kernel_spmd

    in_maps, use_bqk, use_bp = _prepare(**inputs)
    key = (BPC, use_bqk, use_bp)
    if key not in _CACHE:
        _CACHE[key] = _build(*key)
    nc = _CACHE[key]
    res = run_bass_kernel_spmd(nc, in_maps, core_ids=list(range(NCORES)),
                               trace=trace, **run_kwargs)
    y = np.concatenate([res.results[c]["y"] for c in range(NCORES)], axis=0)
    return y, res


def kernel(**inputs):
    y, _ = run(inputs)
    return y


# revision 21
# speedup vs baseline: 1.2180x; 1.0132x over previous
# Self-contained Trainium2 Bass kernel for the LN->QKV->sparse-rel-pos-attention->proj block.
#
# Reference computation (B=128, N=256, DIM=512, H=12, KD=32, D=128):
#   xn   = LayerNorm(x) * gamma + beta
#   qkv  = xn @ Wqkv + bqkv ; split q,k,v per head
#   attn = softmax(q k^T / sqrt(KD) + biases[:, bias_idxs])
#   out  = (attn @ v) @ Wproj + bproj
#
# Strategy: pure data-parallel over batch across 8 NeuronCores (16 elems/core).
# Host folds: gamma/beta into Wqkv, 1/sqrt(KD) into Wq, v-bias into bproj,
# and expands exp(biases[:, bias_idxs]) so softmax(S+B) = expS*expB row-normalized.
# Device layouts avoid all transposes except the initial z -> z^T:
#   qk^T [feat, tok] and v [tok, feat] both come from matmuls against z^T;
#   S^T = k q^T has tokens-m on partitions so exp/Z/AV consume it directly;
#   AV gives O^T [head-dim, tok] which is exactly proj's stationary layout.
# Softmax normalizer: Z is computed partition-broadcast directly by a matmul
# with an all-ones [128,128] stationary (same PE cost as a [1,x] row, but the
# result lands replicated across partitions -> no DRAM-roundtrip broadcast),
# then one DVE approx-reciprocal and one fused multiply.
# q/k features pack into 8 chunks of 128 (strips of 32 at bases 0/32/64 --
# the PE requires stationary base partition in {0,32,64}): head h has q in
# chunk h//3, k in chunk 4+h//3, both at strip rows 32*(h%3). Heads are
# processed in strip-pure pairs because interleaving different PE
# tile_positions on one PSUM bank hangs the device.
# PE p-state: full speed (2.4GHz) only after 3us of continuous busy; every
# idle gap drops it to 1.2GHz for the next 3us. The emission order is
# software-pipelined so the PE rarely waits: v-production matmuls interleave
# with S rounds, Z/AV trail S by 2 rounds (covering the exp->expB-mult
# latency on ACT/DVE/GpSimd), next-pair LayerNorm DVE work is spread through
# this pair's rounds, and next-pair transposes cover proj's wait for the
# last normalize.
# x/y ride DMA with 2 tokens per partition line (4KB contiguous packets;
# token-major 2KB lines only reach ~23GB/s) => attention token order is
# position (s*128+q) <-> token 2q+s; expb is permuted to match on the host.
# Weight-constant DMA descriptors are issued from otherwise-idle engines
# (ident/wqk on Activation, the rest on GpSimd; x on Sync) -- issuing one
# rearranged weight DMA costs ~3.6us of the issuing engine's queue, which
# must not block the first LayerNorm Sqrt / zT copies.

import numpy as np

B, N, DIM = 128, 256, 512
H, KD = 12, 32
D = 128
DH = D * H
RES = 16
EPS = 1e-5
NCORES = 8
BPC = B // NCORES

_CACHE = {}

# heads processed in strip-pure pairs ((h%3) equal within each pair), ordered
# so v chunks (c0: heads 0-3, c1: 4-7, c2: 8-11) are needed progressively
HEAD_ORDER = [0, 3, 1, 4, 2, 5, 6, 9, 7, 10, 8, 11]
# qk chunk emission order: rounds 0-2 need q chunks {0,1}, k {4,5}
FC_ORDER = [0, 1, 4, 5, 2, 3, 6, 7]


def _build(bpc, use_bqk, use_bp):
    from contextlib import ExitStack

    import concourse.bacc as bacc
    import concourse.tile as tile
    from concourse import mybir

    f32 = mybir.dt.float32
    f32r = mybir.dt.float32r
    fp16 = mybir.dt.float16
    Alu = mybir.AluOpType
    Act = mybir.ActivationFunctionType

    nc = bacc.Bacc("TRN2", target_bir_lowering=False, debug=False,
                   num_devices=NCORES)

    x_d = nc.dram_tensor("x", [bpc, N, DIM], f32r, kind="ExternalInput").ap()
    wqk_d = nc.dram_tensor("wqk", [DIM, 8 * 128], f32r, kind="ExternalInput").ap()
    wv_d = nc.dram_tensor("wv", [DIM, DH], f32r, kind="ExternalInput").ap()
    wp_d = nc.dram_tensor("wp", [DH, DIM], f32r, kind="ExternalInput").ap()
    expb_d = nc.dram_tensor("expb", [128, 2, H, N], fp16, kind="ExternalInput").ap()
    ones_d = nc.dram_tensor("ones", [128, 128], f32r, kind="ExternalInput").ap()
    ident_d = nc.dram_tensor("ident", [128, 128], f32r, kind="ExternalInput").ap()
    if use_bqk:
        bqk_d = nc.dram_tensor("bqk", [128, 8], f32, kind="ExternalInput").ap()
    if use_bp:
        bp_d = nc.dram_tensor("bp", [DIM], f32, kind="ExternalInput").ap()
    y_d = nc.dram_tensor("y", [bpc, N, DIM], f32, kind="ExternalOutput").ap()

    NP = bpc // 2

    with tile.TileContext(nc) as tc, ExitStack() as ctx:
        consts = ctx.enter_context(tc.tile_pool(name="consts", bufs=1))
        sb_x = ctx.enter_context(tc.tile_pool(name="sb_x", bufs=2))
        sb_zT = ctx.enter_context(tc.tile_pool(name="sb_zT", bufs=2))
        sb_qkT = ctx.enter_context(tc.tile_pool(name="sb_qkT", bufs=2))
        sb_v = ctx.enter_context(tc.tile_pool(name="sb_v", bufs=2))
        sb_pt = ctx.enter_context(tc.tile_pool(name="sb_pt", bufs=3))
        sb_zb = ctx.enter_context(tc.tile_pool(name="sb_zb", bufs=2))
        sb_ot = ctx.enter_context(tc.tile_pool(name="sb_ot", bufs=2))
        sb_yb = ctx.enter_context(tc.tile_pool(name="sb_yb", bufs=2))
        sb_small = ctx.enter_context(tc.tile_pool(name="sb_small", bufs=3))
        ps_work = ctx.enter_context(tc.tile_pool(name="ps_work", bufs=2, space="PSUM"))
        ps_s = ctx.enter_context(tc.tile_pool(name="ps_s", bufs=2, space="PSUM"))
        ps_ot = ctx.enter_context(tc.tile_pool(name="ps_ot", bufs=2, space="PSUM"))
        ps_z = ctx.enter_context(tc.tile_pool(name="ps_z", bufs=2, space="PSUM"))

        x_tiles = {}

        def issue_x(p):
            t = sb_x.tile([128, 4, DIM], f32r, tag="x", bufs=2)
            nc.sync.dma_start(
                out=t.rearrange("q (e two) d -> q e two d", e=2),
                in_=x_d[2 * p:2 * p + 2].rearrange("e (q two) d -> q e two d",
                                                   q=128))
            x_tiles[p] = t

        def x_slot(p, st):
            t = x_tiles[p]
            if isinstance(t, list):
                return t[st // 2][:, st % 2, :]
            return t[:, st, :]

        # first pair's x goes out first on the sync queue, split per element
        # so el0's LayerNorm/transposes start while el1 is still in flight
        x0 = []
        for el in range(2):
            t = sb_x.tile([128, 2, DIM], f32r, tag="x0", bufs=2)
            nc.sync.dma_start(
                out=t,
                in_=x_d[el].rearrange("(q two) d -> q two d", q=128))
            x0.append(t)
        x_tiles[0] = x0

        # ---- constants (Activation DMA queue, first-use order) ----
        # weights are split and ordered by first use across the two spare
        # DMA queues (scalar, gpsimd); aggregate HBM bandwidth (~330GB/s)
        # is the startup wall, so bytes not needed early must queue behind
        # bytes that are
        ident = consts.tile([128, 128], f32r, tag="ident")
        nc.scalar.dma_start(out=ident, in_=ident_d)
        wqk_r = wqk_d.rearrange("(kc p) f -> p kc f", p=128)
        wqk_h = []
        for hf in range(2):
            t = consts.tile([128, 4, 512], f32r, tag=f"wqkh{hf}")
            nc.scalar.dma_start(out=t, in_=wqk_r[:, :, hf * 512:(hf + 1) * 512])
            wqk_h.append(t)
        wv_r = wv_d.rearrange("(kc p) f -> p kc f", p=128)
        wv_c = []
        for c in range(3):
            t = consts.tile([128, 4, 512], f32r, tag=f"wvc{c}")
            wv_c.append(t)
        nc.gpsimd.dma_start(out=wv_c[0], in_=wv_r[:, :, 0:512])
        nc.gpsimd.dma_start(out=wv_c[1], in_=wv_r[:, :, 512:1024])
        expb_sb = consts.tile([128, 2, H, N], fp16, tag="expb")
        nc.gpsimd.dma_start(out=expb_sb, in_=expb_d)
        nc.gpsimd.dma_start(out=wv_c[2], in_=wv_r[:, :, 1024:1536])
        # x for pair 1: the sync queue is still draining pair 0
        x1 = sb_x.tile([128, 4, DIM], f32r, tag="x", bufs=2)
        nc.gpsimd.dma_start(
            out=x1.rearrange("q (e two) d -> q e two d", e=2),
            in_=x_d[2:4].rearrange("e (q two) d -> q e two d", q=128))
        x_tiles[1] = x1
        ones_sb = consts.tile([128, 128], f32r, tag="ones")
        nc.gpsimd.dma_start(out=ones_sb, in_=ones_d)
        wp_sb = consts.tile([128, H, DIM], f32r, tag="wp")
        nc.gpsimd.dma_start(out=wp_sb, in_=wp_d.rearrange("(h p) f -> p h f", p=128))
        eps_t = consts.tile([128, 1], f32)
        nc.vector.memset(eps_t, EPS)
        if use_bqk:
            bqk_sb = consts.tile([128, 8], f32)
            nc.gpsimd.dma_start(out=bqk_sb, in_=bqk_d)
        if use_bp:
            bp_sb = consts.tile([128, 1, DIM], f32)
            nc.gpsimd.dma_start(out=bp_sb, in_=bp_d.partition_broadcast(128))

        # ---- LayerNorm pieces (emitted spread through the previous pair) ----
        def ln_stats(p, st, mvt):
            stats = sb_small.tile([128, 6], f32, tag="stats", bufs=5)
            nc.vector.bn_stats(stats, x_slot(p, st))
            nc.vector.bn_aggr(mvt[:, st, :], stats)

        def ln_finish(mvt):
            sig = sb_small.tile([128, 4], f32, tag="sig")
            nc.scalar.activation(sig, mvt[:, :, 1], Act.Sqrt, bias=eps_t,
                                 scale=1.0)
            rsig = sb_small.tile([128, 4], f32, tag="rsig")
            nc.vector.reciprocal(rsig, sig)
            return rsig

        def ln_norm(p, st, mvt, rsig):
            xs = x_slot(p, st)
            nc.vector.tensor_scalar(out=xs, in0=xs,
                                    scalar1=mvt[:, st, 0:1],
                                    scalar2=rsig[:, st:st + 1],
                                    op0=Alu.subtract, op1=Alu.mult)

        def transposes(p, zT=None, els=(0, 1)):
            if zT is None:
                zT = sb_zT.tile([128, 4, 2 * N], f32r, tag="zT")
            for st in range(4):
                el, sl = st // 2, st % 2
                if el not in els:
                    continue
                xs = x_slot(p, st)
                zT_ps = ps_work.tile([128, 512], f32r, tag="work")
                for kc in range(4):
                    nc.tensor.transpose(zT_ps[:, kc * 128:(kc + 1) * 128],
                                        xs[:, kc * 128:(kc + 1) * 128],
                                        ident)
                off = el * N + sl * 128
                nc.scalar.activation(zT[:, :, off:off + 128],
                                     zT_ps.rearrange("p (kc t) -> p kc t", kc=4),
                                     Act.Copy)
            return zT

        def qk_prod(zT, split=False):
            qkT = sb_qkT.tile([128, 8, 2 * N], f32r, tag="qkT", bufs=1)
            spans = [(0, 2 * N)] if not split else [(0, N), (N, N)]
            for off, w in spans:
                for i, fc in enumerate(FC_ORDER):
                    qk_ps = ps_work.tile([128, 512], f32, tag="work")
                    wt = wqk_h[fc // 4]
                    for kc in range(4):
                        nc.tensor.matmul(qk_ps[:, :w],
                                         lhsT=wt[:, kc, (fc % 4) * 128:(fc % 4 + 1) * 128],
                                         rhs=zT[:, kc, off:off + w],
                                         start=(kc == 0), stop=(kc == 3))
                    if i % 2 == 0:
                        nc.scalar.activation(qkT[:, fc, off:off + w],
                                             qk_ps[:, :w], Act.Copy)
                    else:
                        nc.vector.tensor_copy(out=qkT[:, fc, off:off + w],
                                              in_=qk_ps[:, :w])
            if use_bqk:
                for fc in range(8):
                    nc.vector.tensor_scalar_add(
                        out=qkT[:, fc, :], in0=qkT[:, fc, :],
                        scalar1=bqk_sb[:, fc:fc + 1])
            return qkT

        def v_mm(zT, v_sb, etok, mc, c):
            v_ps = ps_work.tile([128, 512], f32, tag="work")
            for kc in range(4):
                nc.tensor.matmul(
                    v_ps,
                    lhsT=zT[:, kc, etok + mc * 128:etok + (mc + 1) * 128],
                    rhs=wv_c[c][:, kc, :],
                    start=(kc == 0), stop=(kc == 3))
            if (mc + c) % 2 == 0:
                nc.scalar.activation(v_sb[:, mc, c * 512:(c + 1) * 512], v_ps,
                                     Act.Copy)
            else:
                nc.vector.tensor_copy(out=v_sb[:, mc, c * 512:(c + 1) * 512],
                                      in_=v_ps)

        def s_round(qkT, etok, r):
            pt = sb_pt.tile([128, 2, 2, N], f32r, tag="pt")
            for mc in range(2):
                s_ps = ps_s.tile([128, 512], f32, tag="s")
                for hi in range(2):
                    h = HEAD_ORDER[2 * r + hi]
                    qc = h // 3
                    base = (h % 3) * KD
                    nc.tensor.matmul(
                        s_ps[:, hi * N:(hi + 1) * N],
                        lhsT=qkT[base:base + KD, 4 + qc,
                                 etok + mc * 128:etok + (mc + 1) * 128],
                        rhs=qkT[base:base + KD, qc, etok:etok + N],
                        start=True, stop=True)
                nc.scalar.activation(pt[:, mc],
                                     s_ps.rearrange("p (a n) -> p a n", a=2),
                                     Act.Exp)
                eng = nc.gpsimd if (r + mc) % 2 == 0 else nc.vector
                eng.tensor_tensor(out=pt[:, mc], in0=pt[:, mc],
                                  in1=expb_sb[:, mc, 2 * r:2 * r + 2, :],
                                  op=Alu.mult)
            return pt

        def zav_round(pt, v_sb, ot_sb, r):
            zb_ps = ps_z.tile([128, 512], f32, tag="zb")
            for mc in range(2):
                nc.tensor.matmul(zb_ps,
                                 lhsT=ones_sb,
                                 rhs=pt[:, mc, :, :].rearrange("p a n -> p (a n)"),
                                 start=(mc == 0), stop=(mc == 1))
            zr = sb_zb.tile([128, 2, N], f32, tag="zb")
            nc.vector.reciprocal_approx_fast(
                out=zr, in_=zb_ps.rearrange("p (a n) -> p a n", a=2))
            ot_ps = ps_ot.tile([128, 512], f32, tag="otp")
            for hi in range(2):
                h = HEAD_ORDER[2 * r + hi]
                for mc in range(2):
                    nc.tensor.matmul(
                        ot_ps[:, hi * N:(hi + 1) * N],
                        lhsT=v_sb[:, mc, h * 128:(h + 1) * 128],
                        rhs=pt[:, mc, hi, :],
                        start=(mc == 0), stop=(mc == 1))
            # GpSimd cannot read PSUM; normalize stays on DVE
            nc.vector.tensor_tensor(out=ot_sb[:, 2 * r:2 * r + 2, :],
                              in0=ot_ps.rearrange("p (a n) -> p a n", a=2),
                              in1=zr, op=Alu.mult)

        def proj(ot_sb, e):
            yb = sb_yb.tile([128, 2, DIM], f32, tag="yb")
            for nci in range(2):
                y_ps = ps_work.tile([128, 512], f32, tag="work")
                for slot in range(H):
                    nc.tensor.matmul(
                        y_ps,
                        lhsT=ot_sb[:, slot, nci * 128:(nci + 1) * 128],
                        rhs=wp_sb[:, HEAD_ORDER[slot], :],
                        start=(slot == 0), stop=(slot == H - 1))
                if use_bp:
                    nc.vector.tensor_tensor(out=yb[:, nci, :], in0=y_ps,
                                            in1=bp_sb[:, 0, :], op=Alu.add)
                else:
                    nc.scalar.activation(yb[:, nci, :], y_ps, Act.Copy)
            nc.sync.dma_start(
                out=y_d[e].rearrange("(q two) d -> q two d", q=128), in_=yb)

        # ---- prologue: pair 0's LN + transposes, per element so el0's
        # PE work starts while el1's x is still in flight ----
        assert bpc % 2 == 0
        zT = sb_zT.tile([128, 4, 2 * N], f32r, tag="zT")
        for el in range(2):
            mv_e = sb_small.tile([128, 2, 2], f32, tag="mv")
            for s in range(2):
                stats = sb_small.tile([128, 6], f32, tag="stats", bufs=5)
                nc.vector.bn_stats(stats, x0[el][:, s, :])
                nc.vector.bn_aggr(mv_e[:, s, :], stats)
            sig = sb_small.tile([128, 2], f32, tag="sig")
            nc.scalar.activation(sig, mv_e[:, :, 1], Act.Sqrt, bias=eps_t,
                                 scale=1.0)
            rsig = sb_small.tile([128, 2], f32, tag="rsig")
            nc.vector.reciprocal(rsig, sig)
            for s in range(2):
                xs = x0[el][:, s, :]
                nc.vector.tensor_scalar(out=xs, in0=xs,
                                        scalar1=mv_e[:, s, 0:1],
                                        scalar2=rsig[:, s:s + 1],
                                        op0=Alu.subtract, op1=Alu.mult)
            transposes(0, zT=zT, els=(el,))

        for p in range(NP):
            if p + 1 < NP:
                if p + 1 not in x_tiles:
                    issue_x(p + 1)
                mv_n = sb_small.tile([128, 4, 2], f32, tag="mv")
                rs_holder = [None]
            qkT = qk_prod(zT, split=(p == 0))

            prev = None  # (ot_sb, e) of el0 awaiting proj
            for el in range(2):
                e = 2 * p + el
                etok = el * N
                v_sb = sb_v.tile([128, 2, DH], f32r, tag="v")
                ot_sb = sb_ot.tile([128, H, N], f32r, tag="ot")
                pts = {}

                def hook(i):
                    # spread next pair's LN through el0's rounds. The tile
                    # scheduler orders engine queues by SIMULATED ready time
                    # (it does not respect emission order); pair 1's x lands
                    # much later than the sim thinks, so floor pair-1 LN deep
                    # into the pair-0 queue or it head-of-line-blocks the
                    # prologue's reciprocals on the DVE.
                    if el != 0 or p + 1 >= NP:
                        return
                    from contextlib import nullcontext
                    wait = tc.tile_wait_until(0.04) if p == 0 else nullcontext()
                    with wait:
                        if i < 4:
                            ln_stats(p + 1, i, mv_n)
                        elif i == 4:
                            rs_holder[0] = ln_finish(mv_n)
                            ln_norm(p + 1, 0, mv_n, rs_holder[0])
                            ln_norm(p + 1, 1, mv_n, rs_holder[0])
                        else:
                            ln_norm(p + 1, 2, mv_n, rs_holder[0])
                            ln_norm(p + 1, 3, mv_n, rs_holder[0])

                v_mm(zT, v_sb, etok, 0, 0)
                v_mm(zT, v_sb, etok, 1, 0)
                if prev is not None:
                    proj(*prev)
                    prev = None
                pts[0] = s_round(qkT, etok, 0)
                hook(0)
                v_mm(zT, v_sb, etok, 0, 1)
                pts[1] = s_round(qkT, etok, 1)
                hook(1)
                v_mm(zT, v_sb, etok, 1, 1)
                pts[2] = s_round(qkT, etok, 2)
                zav_round(pts.pop(0), v_sb, ot_sb, 0)
                hook(2)
                v_mm(zT, v_sb, etok, 0, 2)
                pts[3] = s_round(qkT, etok, 3)
                zav_round(pts.pop(1), v_sb, ot_sb, 1)
                hook(3)
                v_mm(zT, v_sb, etok, 1, 2)
                pts[4] = s_round(qkT, etok, 4)
                zav_round(pts.pop(2), v_sb, ot_sb, 2)
                hook(4)
                pts[5] = s_round(qkT, etok, 5)
                zav_round(pts.pop(3), v_sb, ot_sb, 3)
                hook(5)
                zav_round(pts.pop(4), v_sb, ot_sb, 4)
                zav_round(pts.pop(5), v_sb, ot_sb, 5)

                if el == 0:
                    prev = (ot_sb, e)
                else:
                    # next pair's transposes cover proj's wait for the last
                    # normalize
                    if p + 1 < NP:
                        zT = transposes(p + 1)
                    proj(ot_sb, e)

    nc.compile()
    return nc


def _prepare(x, gamma, beta, Wqkv, bqkv, Wproj, bproj, biases, bias_idxs):
    x = np.ascontiguousarray(np.asarray(x, dtype=np.float32))
    gamma = np.asarray(gamma, dtype=np.float32)
    beta = np.asarray(beta, dtype=np.float32)
    Wqkv = np.asarray(Wqkv, dtype=np.float32)
    bqkv = np.asarray(bqkv, dtype=np.float32)
    Wproj = np.asarray(Wproj, dtype=np.float32)
    bproj = np.asarray(bproj, dtype=np.float32)
    biases = np.asarray(biases, dtype=np.float32)
    bias_idxs = np.asarray(bias_idxs)

    s = np.float32(KD ** -0.5)
    Wg = Wqkv * gamma[:, None]
    bfull = beta @ Wqkv + bqkv
    Wr = Wg.reshape(DIM, H, 64 + D)
    br = bfull.reshape(H, 64 + D)
    # feature layout (see kernel comment): head h -> strip h%3; q in chunk
    # h//3, k in chunk 4 + h//3.
    wqk = np.zeros((DIM, 8, 128), dtype=np.float32)
    bqk = np.zeros((8, 128), dtype=np.float32)
    for h in range(H):
        qc, base = h // 3, (h % 3) * KD
        wqk[:, qc, base:base + KD] = Wr[:, h, 0:KD] * s
        wqk[:, 4 + qc, base:base + KD] = Wr[:, h, KD:2 * KD]
        bqk[qc, base:base + KD] = br[h, 0:KD] * s
        bqk[4 + qc, base:base + KD] = br[h, KD:2 * KD]
    wqk = np.ascontiguousarray(wqk.reshape(DIM, 8 * 128))
    wv = np.ascontiguousarray(Wr[:, :, 2 * KD:].reshape(DIM, DH))
    bv = br[:, 2 * KD:].reshape(DH)
    bp = bproj + bv @ Wproj
    expb = np.exp(biases[:, bias_idxs])  # [H, N, N]
    # token positions are interleaved 2-per-partition: pos (s*128+q) <-> token
    # 2q+s; permute both attention axes to match, then reorder heads to the
    # kernel's processing order
    perm = np.arange(N).reshape(128, 2).T.reshape(-1)  # pos -> token
    expb_p = expb[HEAD_ORDER][:, perm][:, :, perm]
    expb_t = np.ascontiguousarray(
        expb_p.reshape(H, 2, 128, N).transpose(2, 1, 0, 3)).astype(np.float16)

    use_bqk = bool(np.abs(bqk).max() > 0)
    use_bp = bool(np.abs(bp).max() > 0)
    bqk_t = np.ascontiguousarray(bqk.T)  # [128, 8]

    common = {"wqk": wqk, "wv": wv, "wp": np.ascontiguousarray(Wproj),
              "expb": expb_t, "ones": np.ones((128, 128), dtype=np.float32),
              "ident": np.eye(128, dtype=np.float32)}
    if use_bqk:
        common["bqk"] = bqk_t
    if use_bp:
        common["bp"] = np.ascontiguousarray(bp)
    in_maps = []
    for c in range(NCORES):
        m = dict(common)
        m["x"] = np.ascontiguousarray(x[c * BPC:(c + 1) * BPC])
        in_maps.append(m)
    return in_maps, use_bqk, use_bp


def run(inputs, trace=False, **run_kwargs):
    from concourse.bass_utils import run_bass_kernel_spmd

    in_maps, use_bqk, use_bp = _prepare(**inputs)
    key = (BPC, use_bqk, use_bp)
    if key not in _CACHE:
        _CACHE[key] = _build(*key)
    nc = _CACHE[key]
    res = run_bass_kernel_spmd(nc, in_maps, core_ids=list(range(NCORES)),
                               trace=trace, **run_kwargs)
    y = np.concatenate([res.results[c]["y"] for c in range(NCORES)], axis=0)
    return y, res


def kernel(**inputs):
    y, _ = run(inputs)
    return y
